# revision 3
# baseline (speedup 1.0000x reference)
"""Bass/Tile kernel for nn_EncoderHead: single-head encoder attention.

Per-core (data-parallel over batch B=8 across 8 NeuronCores):
  x_b [T=2048, C=768], Wq/Wk/Wv [C, H=64], mask_b [1, T] (0 = masked key)
  out_b [T, H] = softmax((x Wq)(x Wk)^T * C**-0.5, masked) @ (x Wv)

Layout strategy (all on-chip after the initial loads):
  - xT [C, T] built by PE transposes of x tiles.
  - qT, kT [H, T] = Wq/Wk^T @ xT  (contraction over c on partitions).
  - vT [H, T] likewise, then PE-transposed to V' [T, H+1] where
    V'[s, 0:H] = v[s,:] * mask[s] and V'[s, H] = mask[s].
  - S^T[s, t] = sum_h kT[h,s] qT[h,t]   (s on partitions, t free).
  - P^T = exp(scale * S^T)  -- no max subtraction needed: logits are O(1)
    (softmax is shift-invariant; reference only shifts for stability).
  - outT'[h', t] = sum_s V'[s, h'] P^T[s, t]  accumulated over s-chunks in
    PSUM; row H is the masked softmax denominator (ones-column trick).
  - transpose outT' back, divide by denominator per-partition, DMA out.
"""

import os
import sys

import numpy as np

_TRN_REPO = "/opt/trn_rl_repo"
if _TRN_REPO not in sys.path and os.path.isdir(_TRN_REPO):
    sys.path.insert(0, _TRN_REPO)

B, T, C, H = 8, 2048, 768, 64
P = 128  # partitions
NT = T // P      # 16 t-chunks of 128
NC = C // P      # 6 c-chunks of 128
NS = T // P      # 16 s-chunks of 128
FT = 512         # matmul moving free-dim tile
NFT = T // FT    # 4 free-dim tiles
SCALE = float(C) ** -0.5

USE_F32R = True  # float32r matmuls: full PE rate at N>=256 (fp32 is 4x slower)

_CACHE = {}


def _build():
    import concourse.bass as bass
    import concourse.tile as tile
    from concourse import bacc, mybir
    from concourse.masks import make_identity

    f32 = mybir.dt.float32
    f32r = mybir.dt.float32r if USE_F32R else mybir.dt.float32
    i32 = mybir.dt.int32
    EXP = mybir.ActivationFunctionType.Exp


    nc = bacc.Bacc(
        "TRN2",
        target_bir_lowering=False,
        debug=False,
        enable_asserts=False,
        num_devices=8,
    )

    x = nc.dram_tensor("x", [T, C], f32, kind="ExternalInput").ap()
    mask = nc.dram_tensor("mask", [1, T], i32, kind="ExternalInput").ap()
    wq = nc.dram_tensor("Wq", [C, H], f32, kind="ExternalInput").ap()
    wk = nc.dram_tensor("Wk", [C, H], f32, kind="ExternalInput").ap()
    wv = nc.dram_tensor("Wv", [C, H], f32, kind="ExternalInput").ap()
    out = nc.dram_tensor("out", [T, H], f32, kind="ExternalOutput").ap()

    with tile.TileContext(nc) as tc:
        from contextlib import ExitStack

        with ExitStack() as ctx:
            const = ctx.enter_context(tc.tile_pool(name="const", bufs=1))

            ident = const.tile([P, P], f32)
            make_identity(nc, ident)

            # Weights as lhsT chunks: w_sb[:, j, :] = W[j*128:(j+1)*128, :]
            w_sbufs = []
            for name, w in (("wq", wq), ("wk", wk), ("wv", wv)):
                w_st = const.tile([P, NC, H], f32, name=f"{name}_st")
                nc.gpsimd.dma_start(
                    out=w_st, in_=w.rearrange("(n p) h -> p n h", p=P)
                )
                w_sb = const.tile([P, NC, H], f32r, name=f"{name}_sb")
                nc.any.tensor_copy(w_sb, w_st)
                w_sbufs.append(w_sb)
            wq_sb, wk_sb, wv_sb = w_sbufs

            # mask as per-partition column per s-chunk: msk_f[p, n] = mask[n*128+p]
            msk_i = const.tile([P, NS], i32)
            nc.gpsimd.dma_start(
                out=msk_i, in_=mask.rearrange("a (n p) -> p (a n)", p=P)
            )
            msk_f = const.tile([P, NS], f32)
            nc.vector.tensor_copy(msk_f, msk_i)

            xT_sb = const.tile([P, NC, T], f32r)      # 48KB/partition
            qT_sb = const.tile([H, T], f32r)
            kT_sb = const.tile([H, T], f32r)
            vp_sb = const.tile([P, NS, H + 1], f32r)  # V' chunks

            # ---- Phase 1: load x and transpose to xT ----
            with ExitStack() as p1:
                xin = p1.enter_context(tc.tile_pool(name="xin", bufs=3))
                pst = p1.enter_context(
                    tc.tile_pool(name="pst", bufs=4, space="PSUM")
                )
                for it in range(NT):
                    x_tile = xin.tile([P, C], f32)
                    nc.sync.dma_start(
                        out=x_tile, in_=x[it * P : (it + 1) * P, :]
                    )
                    for jc in range(NC):
                        pt = pst.tile([P, P], f32)
                        nc.tensor.transpose(
                            pt, x_tile[:, jc * P : (jc + 1) * P], ident
                        )
                        nc.any.tensor_copy(
                            xT_sb[:, jc, it * P : (it + 1) * P], pt
                        )

            # ---- Phase 2: projections qT, kT, vT; build V' ----
            with ExitStack() as p2:
                psp = p2.enter_context(
                    tc.tile_pool(name="psp", bufs=2, space="PSUM")
                )
                vT_sb = p2.enter_context(tc.tile_pool(name="vts", bufs=1)).tile(
                    [H, T], f32
                )
                for w_sb, dst in ((wq_sb, qT_sb), (wk_sb, kT_sb), (wv_sb, vT_sb)):
                    for tt in range(NFT):
                        pp = psp.tile([H, FT], f32)
                        for jc in range(NC):
                            nc.tensor.matmul(
                                pp,
                                lhsT=w_sb[:, jc, :],
                                rhs=xT_sb[:, jc, tt * FT : (tt + 1) * FT],
                                start=(jc == 0),
                                stop=(jc == NC - 1),
                            )
                        nc.any.tensor_copy(dst[:, tt * FT : (tt + 1) * FT], pp)

                psv = p2.enter_context(
                    tc.tile_pool(name="psv", bufs=4, space="PSUM")
                )
                for js in range(NS):
                    pv = psv.tile([P, H], f32)
                    nc.tensor.transpose(
                        pv, vT_sb[:, js * P : (js + 1) * P], ident[0:H, 0:H]
                    )
                    nc.vector.tensor_scalar_mul(
                        vp_sb[:, js, 0:H], pv, msk_f[:, js : js + 1]
                    )
                    nc.any.tensor_copy(
                        vp_sb[:, js, H : H + 1], msk_f[:, js : js + 1]
                    )

            # ---- Phase 3: S^T -> exp -> accumulate outT' ----
            with ExitStack() as p3:
                pso = p3.enter_context(
                    tc.tile_pool(name="pso", bufs=1, space="PSUM")
                )
                out_ps = pso.tile([H + 1, T], f32)  # 4 banks
                with ExitStack() as p3i:
                    pss = p3i.enter_context(
                        tc.tile_pool(name="pss", bufs=4, space="PSUM")
                    )
                    pexp = p3i.enter_context(tc.tile_pool(name="pexp", bufs=3))
                    for js in range(NS):
                        pe = pexp.tile([P, T], f32r)
                        for tt in range(NFT):
                            ps = pss.tile([P, FT], f32)
                            nc.tensor.matmul(
                                ps,
                                lhsT=kT_sb[:, js * P : (js + 1) * P],
                                rhs=qT_sb[:, tt * FT : (tt + 1) * FT],
                                start=True,
                                stop=True,
                            )
                            nc.scalar.activation(
                                pe[:, tt * FT : (tt + 1) * FT], ps, EXP,
                                scale=SCALE,
                            )
                        for tt in range(NFT):
                            nc.tensor.matmul(
                                out_ps[:, tt * FT : (tt + 1) * FT],
                                lhsT=vp_sb[:, js, :],
                                rhs=pe[:, tt * FT : (tt + 1) * FT],
                                start=(js == 0),
                                stop=(js == NS - 1),
                            )

                # ---- Phase 4: normalize + transpose back + store ----
                with ExitStack() as p4:
                    outT_sb = p4.enter_context(
                        tc.tile_pool(name="outts", bufs=1)
                    ).tile([H + 1, T], f32)
                    for tt in range(NFT):
                        nc.any.tensor_copy(
                            outT_sb[:, tt * FT : (tt + 1) * FT],
                            out_ps[:, tt * FT : (tt + 1) * FT],
                        )
                    psf = p4.enter_context(
                        tc.tile_pool(name="psf", bufs=4, space="PSUM")
                    )
                    fin = p4.enter_context(tc.tile_pool(name="fin", bufs=4))
                    oall = p4.enter_context(tc.tile_pool(name="oall", bufs=1)).tile(
                        [P, NT, H], f32
                    )
                    for it in range(NT):
                        pf = psf.tile([P, H + 1], f32)
                        nc.tensor.transpose(
                            pf,
                            outT_sb[:, it * P : (it + 1) * P],
                            ident[0 : H + 1, 0 : H + 1],
                        )
                        rec = fin.tile([P, 1], f32)
                        nc.vector.reciprocal(rec, pf[:, H : H + 1])
                        nc.vector.tensor_scalar_mul(
                            oall[:, it, :], pf[:, 0:H], rec
                        )
                    nc.sync.dma_start(
                        out=out.rearrange("(n p) h -> p n h", p=P), in_=oall
                    )

    nc.compile()
    return nc


def _get_nc():
    if "nc" not in _CACHE:
        _CACHE["nc"] = _build()
    return _CACHE["nc"]


def kernel(**inputs) -> np.ndarray:
    from concourse import bass_utils

    x = np.ascontiguousarray(np.asarray(inputs["x"], dtype=np.float32))
    mask = np.ascontiguousarray(np.asarray(inputs["mask"], dtype=np.int32))
    wq = np.ascontiguousarray(np.asarray(inputs["Wq"], dtype=np.float32))
    wk = np.ascontiguousarray(np.asarray(inputs["Wk"], dtype=np.float32))
    wv = np.ascontiguousarray(np.asarray(inputs["Wv"], dtype=np.float32))

    nc = _get_nc()
    in_maps = [
        {"x": x[b], "mask": mask[b], "Wq": wq, "Wk": wk, "Wv": wv}
        for b in range(B)
    ]
    res = bass_utils.run_bass_kernel_spmd(
        nc, in_maps, core_ids=list(range(B)), **_CACHE.get("run_kwargs", {})
    )
    _CACHE["last_results"] = res
    return np.stack([res.results[b]["out"] for b in range(B)], axis=0)


# revision 5
# speedup vs baseline: 1.1723x; 1.1723x over previous
"""Bass/Tile kernel for nn_EncoderHead: single-head encoder attention.

Per-core (data-parallel over batch B=8 across 8 NeuronCores):
  x_b [T=2048, C=768], Wq/Wk/Wv [C, H=64], mask_b [1, T] (0 = masked key)
  out_b [T, H] = softmax((x Wq)(x Wk)^T * C**-0.5, masked) @ (x Wv)

Layout strategy (all on-chip after the initial loads):
  - xT [C, T] built by PE transposes of x tiles.
  - qT, kT [H, T] = Wq/Wk^T @ xT  (contraction over c on partitions).
  - vT [H, T] likewise, then PE-transposed to V' [T, H+1] where
    V'[s, 0:H] = v[s,:] * mask[s] and V'[s, H] = mask[s].
  - S^T[s, t] = sum_h kT[h,s] qT[h,t]   (s on partitions, t free).
  - P^T = exp(scale * S^T)  -- no max subtraction needed: logits are O(1)
    (softmax is shift-invariant; reference only shifts for stability).
  - outT'[h', t] = sum_s V'[s, h'] P^T[s, t]  accumulated over s-chunks in
    PSUM; row H is the masked softmax denominator (ones-column trick).
  - transpose outT' back, divide by denominator per-partition, DMA out.
"""

import os
import sys

import numpy as np

_TRN_REPO = "/opt/trn_rl_repo"
if _TRN_REPO not in sys.path and os.path.isdir(_TRN_REPO):
    sys.path.insert(0, _TRN_REPO)

B, T, C, H = 8, 2048, 768, 64
P = 128  # partitions
NT = T // P      # 16 t-chunks of 128
NC = C // P      # 6 c-chunks of 128
NS = T // P      # 16 s-chunks of 128
FT = 512         # matmul moving free-dim tile
NFT = T // FT    # 4 free-dim tiles
SCALE = float(C) ** -0.5

USE_F32R = True  # float32r matmuls: full PE rate at N>=256 (fp32 is 4x slower)

_CACHE = {}


def _build():
    import concourse.bass as bass
    import concourse.tile as tile
    from concourse import bacc, mybir
    from concourse.masks import make_identity

    f32 = mybir.dt.float32
    f32r = mybir.dt.float32r if USE_F32R else mybir.dt.float32
    i32 = mybir.dt.int32
    EXP = mybir.ActivationFunctionType.Exp


    nc = bacc.Bacc(
        "TRN2",
        target_bir_lowering=False,
        debug=False,
        enable_asserts=False,
        num_devices=8,
    )

    x = nc.dram_tensor("x", [T, C], f32, kind="ExternalInput").ap()
    mask = nc.dram_tensor("mask", [1, T], i32, kind="ExternalInput").ap()
    wq = nc.dram_tensor("Wq", [C, H], f32, kind="ExternalInput").ap()
    wk = nc.dram_tensor("Wk", [C, H], f32, kind="ExternalInput").ap()
    wv = nc.dram_tensor("Wv", [C, H], f32, kind="ExternalInput").ap()
    out = nc.dram_tensor("out", [T, H], f32, kind="ExternalOutput").ap()

    with tile.TileContext(nc) as tc:
        from contextlib import ExitStack

        with ExitStack() as ctx:
            const = ctx.enter_context(tc.tile_pool(name="const", bufs=1))

            ident = const.tile([P, P], f32)
            make_identity(nc, ident)

            # Weights as lhsT chunks: w_sb[:, j, :] = W[j*128:(j+1)*128, :]
            w_sbufs = []
            for name, w in (("wq", wq), ("wk", wk), ("wv", wv)):
                w_st = const.tile([P, NC, H], f32, name=f"{name}_st")
                nc.gpsimd.dma_start(
                    out=w_st, in_=w.rearrange("(n p) h -> p n h", p=P)
                )
                w_sb = const.tile([P, NC, H], f32r, name=f"{name}_sb")
                nc.any.tensor_copy(w_sb, w_st)
                w_sbufs.append(w_sb)
            wq_sb, wk_sb, wv_sb = w_sbufs

            # mask as per-partition column per s-chunk: msk_f[p, n] = mask[n*128+p]
            msk_i = const.tile([P, NS], i32)
            nc.gpsimd.dma_start(
                out=msk_i, in_=mask.rearrange("a (n p) -> p (a n)", p=P)
            )
            msk_f = const.tile([P, NS], f32)
            nc.vector.tensor_copy(msk_f, msk_i)

            xT_sb = const.tile([P, NC, T], f32r)      # 48KB/partition
            qT_sb = const.tile([H, T], f32r)
            kT_sb = const.tile([H, T], f32r)
            vp_sb = const.tile([P, NS, H + 1], f32r)  # V' chunks

            # ---- Phase 1: load x and transpose to xT ----
            with ExitStack() as p1:
                xin = p1.enter_context(tc.tile_pool(name="xin", bufs=4))
                pst = p1.enter_context(
                    tc.tile_pool(name="pst", bufs=4, space="PSUM")
                )
                for it in range(NT):
                    x_tile = xin.tile([P, C], f32)
                    nc.sync.dma_start(
                        out=x_tile, in_=x[it * P : (it + 1) * P, :]
                    )
                    # 6 transposed [128,128] blocks packed into 2 PSUM banks,
                    # then 2 strided copies into the xT c-planes.
                    for g, nblk in ((0, 4), (4, 2)):
                        pt = pst.tile([P, 512], f32, tag="pt")
                        for b in range(nblk):
                            jc = g + b
                            nc.tensor.transpose(
                                pt[:, b * P : (b + 1) * P],
                                x_tile[:, jc * P : (jc + 1) * P],
                                ident,
                            )
                        nc.any.tensor_copy(
                            xT_sb[:, g : g + nblk, it * P : (it + 1) * P],
                            pt[:, 0 : nblk * P].rearrange(
                                "p (c t) -> p c t", c=nblk
                            ),
                        )

            # ---- Phase 2: projections qT, kT, vT; build V' ----
            with ExitStack() as p2:
                psp = p2.enter_context(
                    tc.tile_pool(name="psp", bufs=2, space="PSUM")
                )
                vT_sb = p2.enter_context(tc.tile_pool(name="vts", bufs=1)).tile(
                    [H, T], f32
                )
                for w_sb, dst in ((wq_sb, qT_sb), (wk_sb, kT_sb), (wv_sb, vT_sb)):
                    for tt in range(NFT):
                        pp = psp.tile([H, FT], f32)
                        for jc in range(NC):
                            nc.tensor.matmul(
                                pp,
                                lhsT=w_sb[:, jc, :],
                                rhs=xT_sb[:, jc, tt * FT : (tt + 1) * FT],
                                start=(jc == 0),
                                stop=(jc == NC - 1),
                            )
                        nc.any.tensor_copy(dst[:, tt * FT : (tt + 1) * FT], pp)

                psv = p2.enter_context(
                    tc.tile_pool(name="psv", bufs=4, space="PSUM")
                )
                for js in range(NS):
                    pv = psv.tile([P, H], f32)
                    nc.tensor.transpose(
                        pv, vT_sb[:, js * P : (js + 1) * P], ident[0:H, 0:H]
                    )
                    nc.vector.tensor_scalar_mul(
                        vp_sb[:, js, 0:H], pv, msk_f[:, js : js + 1]
                    )
                    nc.any.tensor_copy(
                        vp_sb[:, js, H : H + 1], msk_f[:, js : js + 1]
                    )

            # ---- Phase 3: S^T -> exp -> accumulate outT' ----
            with ExitStack() as p3:
                pso = p3.enter_context(
                    tc.tile_pool(name="pso", bufs=1, space="PSUM")
                )
                out_ps = pso.tile([H + 1, T], f32)  # 4 banks
                with ExitStack() as p3i:
                    pss = p3i.enter_context(
                        tc.tile_pool(name="pss", bufs=2, space="PSUM")
                    )
                    pexp = p3i.enter_context(tc.tile_pool(name="pexp", bufs=3))
                    for js in range(NS):
                        pe = pexp.tile([P, T], f32r)
                        for h in range(2):  # two [128,1024] halves
                            ps = pss.tile([P, 2 * FT], f32)
                            for u in range(2):
                                tt = 2 * h + u
                                nc.tensor.matmul(
                                    ps[:, u * FT : (u + 1) * FT],
                                    lhsT=kT_sb[:, js * P : (js + 1) * P],
                                    rhs=qT_sb[:, tt * FT : (tt + 1) * FT],
                                    start=True,
                                    stop=True,
                                )
                            nc.scalar.activation(
                                pe[:, h * 2 * FT : (h + 1) * 2 * FT], ps, EXP,
                                scale=SCALE,
                            )
                        for tt in range(NFT):
                            nc.tensor.matmul(
                                out_ps[:, tt * FT : (tt + 1) * FT],
                                lhsT=vp_sb[:, js, :],
                                rhs=pe[:, tt * FT : (tt + 1) * FT],
                                start=(js == 0),
                                stop=(js == NS - 1),
                            )

                # ---- Phase 4: normalize + transpose back + store ----
                with ExitStack() as p4:
                    outT_sb = p4.enter_context(
                        tc.tile_pool(name="outts", bufs=1)
                    ).tile([H + 1, T], f32)
                    for tt in range(NFT):
                        nc.any.tensor_copy(
                            outT_sb[:, tt * FT : (tt + 1) * FT],
                            out_ps[:, tt * FT : (tt + 1) * FT],
                        )
                    psf = p4.enter_context(
                        tc.tile_pool(name="psf", bufs=4, space="PSUM")
                    )
                    fin = p4.enter_context(tc.tile_pool(name="fin", bufs=4))
                    oall = p4.enter_context(tc.tile_pool(name="oall", bufs=1)).tile(
                        [P, NT, H], f32
                    )
                    for it in range(NT):
                        pf = psf.tile([P, H + 1], f32)
                        nc.tensor.transpose(
                            pf,
                            outT_sb[:, it * P : (it + 1) * P],
                            ident[0 : H + 1, 0 : H + 1],
                        )
                        rec = fin.tile([P, 1], f32)
                        nc.vector.reciprocal(rec, pf[:, H : H + 1])
                        nc.vector.tensor_scalar_mul(
                            oall[:, it, :], pf[:, 0:H], rec
                        )
                    nc.sync.dma_start(
                        out=out.rearrange("(n p) h -> p n h", p=P), in_=oall
                    )

    nc.compile()
    return nc


def _get_nc():
    if "nc" not in _CACHE:
        _CACHE["nc"] = _build()
    return _CACHE["nc"]


def kernel(**inputs) -> np.ndarray:
    from concourse import bass_utils

    x = np.ascontiguousarray(np.asarray(inputs["x"], dtype=np.float32))
    mask = np.ascontiguousarray(np.asarray(inputs["mask"], dtype=np.int32))
    wq = np.ascontiguousarray(np.asarray(inputs["Wq"], dtype=np.float32))
    wk = np.ascontiguousarray(np.asarray(inputs["Wk"], dtype=np.float32))
    wv = np.ascontiguousarray(np.asarray(inputs["Wv"], dtype=np.float32))

    nc = _get_nc()
    in_maps = [
        {"x": x[b], "mask": mask[b], "Wq": wq, "Wk": wk, "Wv": wv}
        for b in range(B)
    ]
    res = bass_utils.run_bass_kernel_spmd(
        nc, in_maps, core_ids=list(range(B)), **_CACHE.get("run_kwargs", {})
    )
    _CACHE["last_results"] = res
    return np.stack([res.results[b]["out"] for b in range(B)], axis=0)


# revision 7
# speedup vs baseline: 1.2879x; 1.0986x over previous
"""Bass/Tile kernel for nn_EncoderHead: single-head encoder attention.

Per-core (data-parallel over batch B=8 across 8 NeuronCores):
  x_b [T=2048, C=768], Wq/Wk/Wv [C, H=64], mask_b [1, T] (0 = masked key)
  out_b [T, H] = softmax((x Wq)(x Wk)^T * C**-0.5, masked) @ (x Wv)

Layout strategy (all on-chip after the initial loads):
  - xT [C, T] built by PE transposes of x tiles.
  - qT, kT [H, T] = [Wq|Wk]^T @ xT in one packed matmul chain (contraction
    over c on partitions, 128 stationary columns).
  - vT [H, T] likewise, then PE-transposed to V' [T, H+1] where
    V'[s, 0:H] = v[s,:] * mask[s] and V'[s, H] = mask[s].
  - S^T[s, t] = sum_h kT[h,s] qT[h,t]   (s on partitions, t free).
  - P^T = exp(scale * S^T)  -- no max subtraction needed: logits are O(1)
    (softmax is shift-invariant; reference only shifts for stability).
  - outT'[h', t] = sum_s V'[s, h'] P^T[s, t]  accumulated over s-chunks in
    PSUM; row H is the masked softmax denominator (ones-column trick).
  - transpose outT' back, divide by denominator per-partition, DMA out.

Scheduling: emission order is software-pipelined (Tile keeps per-engine FIFO
order): x-transposes of group g+1 are emitted before projections of group g,
and S^T+exp of s-chunk js+1 before the PV matmuls of js, so the PE stream
stays dense and ACT (exp) runs back-to-back.
"""

import os
import sys

import numpy as np

_TRN_REPO = "/opt/trn_rl_repo"
if _TRN_REPO not in sys.path and os.path.isdir(_TRN_REPO):
    sys.path.insert(0, _TRN_REPO)

B, T, C, H = 8, 2048, 768, 64
P = 128  # partitions
NT = T // P      # 16 t-chunks of 128
NC = C // P      # 6 c-chunks of 128
NS = T // P      # 16 s-chunks of 128
FT = 512         # matmul moving free-dim tile
NFT = T // FT    # 4 free-dim tiles
SCALE = float(C) ** -0.5

_CACHE = {}


def _build():
    from contextlib import ExitStack

    import concourse.bass as bass  # noqa: F401
    import concourse.tile as tile
    from concourse import bacc, mybir
    from concourse.masks import make_identity

    f32 = mybir.dt.float32
    f32r = mybir.dt.float32r
    i32 = mybir.dt.int32
    EXP = mybir.ActivationFunctionType.Exp

    nc = bacc.Bacc(
        "TRN2",
        target_bir_lowering=False,
        debug=False,
        enable_asserts=False,
        num_devices=8,
    )

    x = nc.dram_tensor("x", [T, C], f32, kind="ExternalInput").ap()
    mask = nc.dram_tensor("mask", [1, T], i32, kind="ExternalInput").ap()
    wq = nc.dram_tensor("Wq", [C, H], f32, kind="ExternalInput").ap()
    wk = nc.dram_tensor("Wk", [C, H], f32, kind="ExternalInput").ap()
    wv = nc.dram_tensor("Wv", [C, H], f32, kind="ExternalInput").ap()
    out = nc.dram_tensor("out", [T, H], f32, kind="ExternalOutput").ap()

    with tile.TileContext(nc) as tc, ExitStack() as ctx:
        const = ctx.enter_context(tc.tile_pool(name="const", bufs=1))

        ident = const.tile([P, P], f32)
        make_identity(nc, ident)

        # Packed [Wq | Wk] stationary chunks: wqk_sb[:, j, 0:H] = Wq chunk j,
        # [:, j, H:2H] = Wk chunk j. One matmul chain produces qT and kT.
        wqk_st = const.tile([P, NC, 2 * H], f32)
        nc.scalar.dma_start(
            out=wqk_st[:, :, 0:H], in_=wq.rearrange("(n p) h -> p n h", p=P)
        )
        nc.scalar.dma_start(
            out=wqk_st[:, :, H : 2 * H],
            in_=wk.rearrange("(n p) h -> p n h", p=P),
        )
        wqk_sb = const.tile([P, NC, 2 * H], f32r)
        nc.vector.tensor_copy(wqk_sb, wqk_st)

        wv_st = const.tile([P, NC, H], f32)
        nc.scalar.dma_start(
            out=wv_st, in_=wv.rearrange("(n p) h -> p n h", p=P)
        )
        wv_sb = const.tile([P, NC, H], f32r)
        nc.vector.tensor_copy(wv_sb, wv_st)

        # mask as per-partition column per s-chunk: msk_f[p, n] = mask[n*P+p]
        msk_i = const.tile([P, NS], i32)
        nc.gpsimd.dma_start(
            out=msk_i, in_=mask.rearrange("a (n p) -> p (a n)", p=P)
        )
        msk_f = const.tile([P, NS], f32)
        nc.vector.tensor_copy(msk_f, msk_i)

        xT_sb = const.tile([P, NC, T], f32r)      # 48KB/partition
        qT_sb = const.tile([H, T], f32r)
        kT_sb = const.tile([H, T], f32r)
        vp_sb = const.tile([P, NS, H + 1], f32r)  # V' chunks

        # ---- Phase 1+2: load x, transpose, project, build V' ----
        with ExitStack() as p12:
            xin = p12.enter_context(tc.tile_pool(name="xin", bufs=4))
            pst = p12.enter_context(
                tc.tile_pool(name="pst", bufs=4, space="PSUM")
            )
            psp = p12.enter_context(
                tc.tile_pool(name="psp", bufs=2, space="PSUM")
            )
            psv = p12.enter_context(
                tc.tile_pool(name="psv", bufs=2, space="PSUM")
            )
            vT_sb = p12.enter_context(tc.tile_pool(name="vts", bufs=1)).tile(
                [H, T], f32
            )

            def emit_chunks(g):
                for k in range(4):
                    it = 4 * g + k
                    x_tile = xin.tile([P, C], f32, name="x_tile")
                    nc.sync.dma_start(
                        out=x_tile, in_=x[it * P : (it + 1) * P, :]
                    )
                    # 6 transposed [128,128] blocks packed into 2 PSUM banks,
                    # then 2 strided copies into the xT c-planes.
                    for gg, nblk in ((0, 4), (4, 2)):
                        pt = pst.tile([P, 512], f32, tag="pt", name="pt")
                        for b in range(nblk):
                            jc = gg + b
                            nc.tensor.transpose(
                                pt[:, b * P : (b + 1) * P],
                                x_tile[:, jc * P : (jc + 1) * P],
                                ident,
                            )
                        nc.any.tensor_copy(
                            xT_sb[:, gg : gg + nblk, it * P : (it + 1) * P],
                            pt[:, 0 : nblk * P].rearrange(
                                "p (c t) -> p c t", c=nblk
                            ),
                        )

            def emit_proj(tt):
                # q,k packed: psum rows 0:H = qT tile, H:2H = kT tile
                pp = psp.tile([P, FT], f32, tag="pp", name="pp")
                for jc in range(NC):
                    nc.tensor.matmul(
                        pp,
                        lhsT=wqk_sb[:, jc, :],
                        rhs=xT_sb[:, jc, tt * FT : (tt + 1) * FT],
                        start=(jc == 0),
                        stop=(jc == NC - 1),
                    )
                nc.any.tensor_copy(
                    qT_sb[:, tt * FT : (tt + 1) * FT], pp[0:H, :]
                )
                nc.any.tensor_copy(
                    kT_sb[:, tt * FT : (tt + 1) * FT], pp[H : 2 * H, :]
                )
                pv_ps = psp.tile([H, FT], f32, tag="pp", name="pv_ps")
                for jc in range(NC):
                    nc.tensor.matmul(
                        pv_ps,
                        lhsT=wv_sb[:, jc, :],
                        rhs=xT_sb[:, jc, tt * FT : (tt + 1) * FT],
                        start=(jc == 0),
                        stop=(jc == NC - 1),
                    )
                nc.any.tensor_copy(
                    vT_sb[:, tt * FT : (tt + 1) * FT], pv_ps
                )
                # V' chunks for this t-window: v rows * mask, plus the
                # mask ones-column (masked softmax denominator trick).
                for k in range(4):
                    js = 4 * tt + k
                    pv = psv.tile([P, H], f32, tag="pv", name="pv")
                    nc.tensor.transpose(
                        pv, vT_sb[:, js * P : (js + 1) * P], ident[0:H, 0:H]
                    )
                    nc.vector.tensor_scalar_mul(
                        vp_sb[:, js, 0:H], pv, msk_f[:, js : js + 1]
                    )
                    nc.vector.tensor_copy(
                        vp_sb[:, js, H : H + 1], msk_f[:, js : js + 1]
                    )

            # software pipeline: transposes one group ahead of projections
            for g in range(NFT + 1):
                if g < NFT:
                    emit_chunks(g)
                if g >= 1:
                    emit_proj(g - 1)

        # ---- Phase 3: S^T -> exp -> accumulate outT' ----
        with ExitStack() as p3:
            pso = p3.enter_context(
                tc.tile_pool(name="pso", bufs=1, space="PSUM")
            )
            out_ps = pso.tile([H + 1, T], f32)  # 4 banks
            with ExitStack() as p3i:
                pss = p3i.enter_context(
                    tc.tile_pool(name="pss", bufs=2, space="PSUM")
                )
                pexp = p3i.enter_context(tc.tile_pool(name="pexp", bufs=3))
                pe_tiles = {}

                def st_unit(js):
                    pe = pexp.tile([P, T], f32r, tag="pe", name="pe")
                    pe_tiles[js] = pe
                    for h in range(2):  # two [128,1024] halves
                        ps = pss.tile([P, 2 * FT], f32, tag="ps", name="ps")
                        for u in range(2):
                            tt = 2 * h + u
                            nc.tensor.matmul(
                                ps[:, u * FT : (u + 1) * FT],
                                lhsT=kT_sb[:, js * P : (js + 1) * P],
                                rhs=qT_sb[:, tt * FT : (tt + 1) * FT],
                                start=True,
                                stop=True,
                            )
                        nc.scalar.activation(
                            pe[:, h * 2 * FT : (h + 1) * 2 * FT], ps, EXP,
                            scale=SCALE,
                        )

                def pv_unit(js):
                    pe = pe_tiles.pop(js)
                    for tt in range(NFT):
                        nc.tensor.matmul(
                            out_ps[:, tt * FT : (tt + 1) * FT],
                            lhsT=vp_sb[:, js, :],
                            rhs=pe[:, tt * FT : (tt + 1) * FT],
                            start=(js == 0),
                            stop=(js == NS - 1),
                        )

                st_unit(0)
                for js in range(NS):
                    if js + 1 < NS:
                        st_unit(js + 1)
                    pv_unit(js)

            # ---- Phase 4: normalize + transpose back + store ----
            with ExitStack() as p4:
                outT_sb = p4.enter_context(
                    tc.tile_pool(name="outts", bufs=1)
                ).tile([H + 1, T], f32)
                for tt in range(NFT):
                    nc.any.tensor_copy(
                        outT_sb[:, tt * FT : (tt + 1) * FT],
                        out_ps[:, tt * FT : (tt + 1) * FT],
                    )
                psf = p4.enter_context(
                    tc.tile_pool(name="psf", bufs=4, space="PSUM")
                )
                fin = p4.enter_context(tc.tile_pool(name="fin", bufs=4))
                oall = p4.enter_context(tc.tile_pool(name="oall", bufs=1)).tile(
                    [P, NT, H], f32
                )
                for it in range(NT):
                    pf = psf.tile([P, H + 1], f32, tag="pf", name="pf")
                    nc.tensor.transpose(
                        pf,
                        outT_sb[:, it * P : (it + 1) * P],
                        ident[0 : H + 1, 0 : H + 1],
                    )
                    rec = fin.tile([P, 1], f32, tag="rec", name="rec")
                    nc.vector.reciprocal(rec, pf[:, H : H + 1])
                    nc.vector.tensor_scalar_mul(
                        oall[:, it, :], pf[:, 0:H], rec
                    )
                nc.sync.dma_start(
                    out=out.rearrange("(n p) h -> p n h", p=P), in_=oall
                )

    nc.compile()
    return nc


def _get_nc():
    if "nc" not in _CACHE:
        _CACHE["nc"] = _build()
    return _CACHE["nc"]


def kernel(**inputs) -> np.ndarray:
    from concourse import bass_utils

    x = np.ascontiguousarray(np.asarray(inputs["x"], dtype=np.float32))
    mask = np.ascontiguousarray(np.asarray(inputs["mask"], dtype=np.int32))
    wq = np.ascontiguousarray(np.asarray(inputs["Wq"], dtype=np.float32))
    wk = np.ascontiguousarray(np.asarray(inputs["Wk"], dtype=np.float32))
    wv = np.ascontiguousarray(np.asarray(inputs["Wv"], dtype=np.float32))

    nc = _get_nc()
    in_maps = [
        {"x": x[b], "mask": mask[b], "Wq": wq, "Wk": wk, "Wv": wv}
        for b in range(B)
    ]
    res = bass_utils.run_bass_kernel_spmd(
        nc, in_maps, core_ids=list(range(B)), **_CACHE.get("run_kwargs", {})
    )
    _CACHE["last_results"] = res
    return np.stack([res.results[b]["out"] for b in range(B)], axis=0)


# revision 8
# speedup vs baseline: 1.2911x; 1.0025x over previous
"""Bass/Tile kernel for nn_EncoderHead: single-head encoder attention.

Per-core (data-parallel over batch B=8 across 8 NeuronCores):
  x_b [T=2048, C=768], Wq/Wk/Wv [C, H=64], mask_b [1, T] (0 = masked key)
  out_b [T, H] = softmax((x Wq)(x Wk)^T * C**-0.5, masked) @ (x Wv)

Layout strategy (all on-chip after the initial loads):
  - xT [C, T] built by PE transposes of x tiles.
  - qT, kT [H, T] = [Wq|Wk]^T @ xT in one packed matmul chain (contraction
    over c on partitions, 128 stationary columns).
  - vT [H, T] likewise, then PE-transposed to V' [T, H+1] where
    V'[s, 0:H] = v[s,:] * mask[s] and V'[s, H] = mask[s].
  - S^T[s, t] = sum_h kT[h,s] qT[h,t]   (s on partitions, t free).
  - P^T = exp(scale * S^T)  -- no max subtraction needed: logits are O(1)
    (softmax is shift-invariant; reference only shifts for stability).
  - outT'[h', t] = sum_s V'[s, h'] P^T[s, t]  accumulated over s-chunks in
    PSUM; row H is the masked softmax denominator (ones-column trick).
  - transpose outT' back, divide by denominator per-partition, DMA out.

Scheduling: emission order is software-pipelined (Tile keeps per-engine FIFO
order): x-transposes of group g+1 are emitted before projections of group g,
and S^T+exp of s-chunk js+1 before the PV matmuls of js, so the PE stream
stays dense and ACT (exp) runs back-to-back.
"""

import os
import sys

import numpy as np

_TRN_REPO = "/opt/trn_rl_repo"
if _TRN_REPO not in sys.path and os.path.isdir(_TRN_REPO):
    sys.path.insert(0, _TRN_REPO)

B, T, C, H = 8, 2048, 768, 64
P = 128  # partitions
NT = T // P      # 16 t-chunks of 128
NC = C // P      # 6 c-chunks of 128
NS = T // P      # 16 s-chunks of 128
FT = 512         # matmul moving free-dim tile
NFT = T // FT    # 4 free-dim tiles
SCALE = float(C) ** -0.5

_CACHE = {}


def _build():
    from contextlib import ExitStack

    import concourse.bass as bass  # noqa: F401
    import concourse.tile as tile
    from concourse import bacc, mybir
    from concourse.masks import make_identity

    f32 = mybir.dt.float32
    f32r = mybir.dt.float32r
    i32 = mybir.dt.int32
    EXP = mybir.ActivationFunctionType.Exp

    nc = bacc.Bacc(
        "TRN2",
        target_bir_lowering=False,
        debug=False,
        enable_asserts=False,
        num_devices=8,
    )

    x = nc.dram_tensor("x", [T, C], f32, kind="ExternalInput").ap()
    mask = nc.dram_tensor("mask", [1, T], i32, kind="ExternalInput").ap()
    wq = nc.dram_tensor("Wq", [C, H], f32, kind="ExternalInput").ap()
    wk = nc.dram_tensor("Wk", [C, H], f32, kind="ExternalInput").ap()
    wv = nc.dram_tensor("Wv", [C, H], f32, kind="ExternalInput").ap()
    out = nc.dram_tensor("out", [T, H], f32, kind="ExternalOutput").ap()

    with tile.TileContext(nc) as tc, ExitStack() as ctx:
        const = ctx.enter_context(tc.tile_pool(name="const", bufs=1))

        ident = const.tile([P, P], f32)
        make_identity(nc, ident)

        # Packed [Wq | Wk] stationary chunks: wqk_sb[:, j, 0:H] = Wq chunk j,
        # [:, j, H:2H] = Wk chunk j. One matmul chain produces qT and kT.
        wqk_st = const.tile([P, NC, 2 * H], f32)
        nc.scalar.dma_start(
            out=wqk_st[:, :, 0:H], in_=wq.rearrange("(n p) h -> p n h", p=P)
        )
        nc.scalar.dma_start(
            out=wqk_st[:, :, H : 2 * H],
            in_=wk.rearrange("(n p) h -> p n h", p=P),
        )
        wqk_sb = const.tile([P, NC, 2 * H], f32r)
        nc.vector.tensor_copy(wqk_sb, wqk_st)

        wv_st = const.tile([P, NC, H], f32)
        nc.scalar.dma_start(
            out=wv_st, in_=wv.rearrange("(n p) h -> p n h", p=P)
        )
        wv_sb = const.tile([P, NC, H], f32r)
        nc.vector.tensor_copy(wv_sb, wv_st)

        # mask as per-partition column per s-chunk: msk_f[p, n] = mask[n*P+p]
        msk_i = const.tile([P, NS], i32)
        nc.gpsimd.dma_start(
            out=msk_i, in_=mask.rearrange("a (n p) -> p (a n)", p=P)
        )
        msk_f = const.tile([P, NS], f32)
        nc.vector.tensor_copy(msk_f, msk_i)

        xT_sb = const.tile([P, NC, T], f32r)      # 48KB/partition
        qT_sb = const.tile([H, T], f32r)
        kT_sb = const.tile([H, T], f32r)
        vp_sb = const.tile([P, NS, H + 1], f32r)  # V' chunks

        # ---- Phase 1+2: load x, transpose, project, build V' ----
        with ExitStack() as p12:
            xin = p12.enter_context(tc.tile_pool(name="xin", bufs=4))
            pst = p12.enter_context(
                tc.tile_pool(name="pst", bufs=4, space="PSUM")
            )
            psp = p12.enter_context(
                tc.tile_pool(name="psp", bufs=2, space="PSUM")
            )
            psv = p12.enter_context(
                tc.tile_pool(name="psv", bufs=2, space="PSUM")
            )
            vT_sb = p12.enter_context(tc.tile_pool(name="vts", bufs=1)).tile(
                [H, T], f32
            )

            def emit_chunks(g):
                for k in range(4):
                    it = 4 * g + k
                    x_tile = xin.tile([P, C], f32, name="x_tile")
                    nc.sync.dma_start(
                        out=x_tile, in_=x[it * P : (it + 1) * P, :]
                    )
                    # 6 transposed [128,128] blocks packed into 2 PSUM banks,
                    # then 2 strided copies into the xT c-planes.
                    for gg, nblk in ((0, 4), (4, 2)):
                        pt = pst.tile([P, 512], f32, tag="pt", name="pt")
                        for b in range(nblk):
                            jc = gg + b
                            nc.tensor.transpose(
                                pt[:, b * P : (b + 1) * P],
                                x_tile[:, jc * P : (jc + 1) * P],
                                ident,
                            )
                        nc.any.tensor_copy(
                            xT_sb[:, gg : gg + nblk, it * P : (it + 1) * P],
                            pt[:, 0 : nblk * P].rearrange(
                                "p (c t) -> p c t", c=nblk
                            ),
                        )

            def emit_proj(tt):
                # q,k packed: psum rows 0:H = qT tile, H:2H = kT tile
                pp = psp.tile([P, FT], f32, tag="pp", name="pp")
                for jc in range(NC):
                    nc.tensor.matmul(
                        pp,
                        lhsT=wqk_sb[:, jc, :],
                        rhs=xT_sb[:, jc, tt * FT : (tt + 1) * FT],
                        start=(jc == 0),
                        stop=(jc == NC - 1),
                    )
                nc.any.tensor_copy(
                    qT_sb[:, tt * FT : (tt + 1) * FT], pp[0:H, :]
                )
                nc.any.tensor_copy(
                    kT_sb[:, tt * FT : (tt + 1) * FT], pp[H : 2 * H, :]
                )
                pv_ps = psp.tile([H, FT], f32, tag="pp", name="pv_ps")
                for jc in range(NC):
                    nc.tensor.matmul(
                        pv_ps,
                        lhsT=wv_sb[:, jc, :],
                        rhs=xT_sb[:, jc, tt * FT : (tt + 1) * FT],
                        start=(jc == 0),
                        stop=(jc == NC - 1),
                    )
                nc.any.tensor_copy(
                    vT_sb[:, tt * FT : (tt + 1) * FT], pv_ps
                )
                # V' chunks for this t-window: v rows * mask, plus the
                # mask ones-column (masked softmax denominator trick).
                for k in range(4):
                    js = 4 * tt + k
                    pv = psv.tile([P, H], f32, tag="pv", name="pv")
                    nc.tensor.transpose(
                        pv, vT_sb[:, js * P : (js + 1) * P], ident[0:H, 0:H]
                    )
                    nc.vector.tensor_scalar_mul(
                        vp_sb[:, js, 0:H], pv, msk_f[:, js : js + 1]
                    )
                    nc.vector.tensor_copy(
                        vp_sb[:, js, H : H + 1], msk_f[:, js : js + 1]
                    )

            # software pipeline: transposes one group ahead of projections
            for g in range(NFT + 1):
                if g < NFT:
                    emit_chunks(g)
                if g >= 1:
                    emit_proj(g - 1)

        # ---- Phase 3: S^T -> exp -> accumulate outT' ----
        with ExitStack() as p3:
            pso = p3.enter_context(
                tc.tile_pool(name="pso", bufs=1, space="PSUM")
            )
            out_ps = pso.tile([H + 1, T], f32)  # 4 banks
            with ExitStack() as p3i:
                pss = p3i.enter_context(
                    tc.tile_pool(name="pss", bufs=2, space="PSUM")
                )
                pexp = p3i.enter_context(tc.tile_pool(name="pexp", bufs=3))
                from concourse.tile import add_dep_helper

                pe_tiles = {}
                st_last = {}

                def st_unit(js):
                    pe = pexp.tile([P, T], f32r, tag="pe", name="pe")
                    pe_tiles[js] = pe
                    for h in range(2):  # two [128,1024] halves
                        ps = pss.tile([P, 2 * FT], f32, tag="ps", name="ps")
                        for u in range(2):
                            tt = 2 * h + u
                            mm = nc.tensor.matmul(
                                ps[:, u * FT : (u + 1) * FT],
                                lhsT=kT_sb[:, js * P : (js + 1) * P],
                                rhs=qT_sb[:, tt * FT : (tt + 1) * FT],
                                start=True,
                                stop=True,
                            )
                        nc.scalar.activation(
                            pe[:, h * 2 * FT : (h + 1) * 2 * FT], ps, EXP,
                            scale=SCALE,
                        )
                    st_last[js] = mm

                def pv_unit(js):
                    pe = pe_tiles.pop(js)
                    gate = st_last.pop(js + 1, None)
                    for tt in range(NFT):
                        mm = nc.tensor.matmul(
                            out_ps[:, tt * FT : (tt + 1) * FT],
                            lhsT=vp_sb[:, js, :],
                            rhs=pe[:, tt * FT : (tt + 1) * FT],
                            start=(js == 0),
                            stop=(js == NS - 1),
                        )
                        if gate is not None:
                            # ordering-only edge: keep the PE queue doing
                            # S^T(js+1) before PV(js) so exp never starves
                            add_dep_helper(
                                mm.ins, gate.ins, sync=False,
                                reason="phase3 sw-pipeline order",
                            )

                st_unit(0)
                for js in range(NS):
                    if js + 1 < NS:
                        st_unit(js + 1)
                    pv_unit(js)

            # ---- Phase 4: normalize + transpose back + store ----
            with ExitStack() as p4:
                outT_sb = p4.enter_context(
                    tc.tile_pool(name="outts", bufs=1)
                ).tile([H + 1, T], f32)
                for tt in range(NFT):
                    nc.any.tensor_copy(
                        outT_sb[:, tt * FT : (tt + 1) * FT],
                        out_ps[:, tt * FT : (tt + 1) * FT],
                    )
                psf = p4.enter_context(
                    tc.tile_pool(name="psf", bufs=4, space="PSUM")
                )
                fin = p4.enter_context(tc.tile_pool(name="fin", bufs=4))
                oall = p4.enter_context(tc.tile_pool(name="oall", bufs=1)).tile(
                    [P, NT, H], f32
                )
                for it in range(NT):
                    pf = psf.tile([P, H + 1], f32, tag="pf", name="pf")
                    nc.tensor.transpose(
                        pf,
                        outT_sb[:, it * P : (it + 1) * P],
                        ident[0 : H + 1, 0 : H + 1],
                    )
                    rec = fin.tile([P, 1], f32, tag="rec", name="rec")
                    nc.vector.reciprocal(rec, pf[:, H : H + 1])
                    nc.vector.tensor_scalar_mul(
                        oall[:, it, :], pf[:, 0:H], rec
                    )
                nc.sync.dma_start(
                    out=out.rearrange("(n p) h -> p n h", p=P), in_=oall
                )

    nc.compile()
    return nc


def _get_nc():
    if "nc" not in _CACHE:
        _CACHE["nc"] = _build()
    return _CACHE["nc"]


def kernel(**inputs) -> np.ndarray:
    from concourse import bass_utils

    x = np.ascontiguousarray(np.asarray(inputs["x"], dtype=np.float32))
    mask = np.ascontiguousarray(np.asarray(inputs["mask"], dtype=np.int32))
    wq = np.ascontiguousarray(np.asarray(inputs["Wq"], dtype=np.float32))
    wk = np.ascontiguousarray(np.asarray(inputs["Wk"], dtype=np.float32))
    wv = np.ascontiguousarray(np.asarray(inputs["Wv"], dtype=np.float32))

    nc = _get_nc()
    in_maps = [
        {"x": x[b], "mask": mask[b], "Wq": wq, "Wk": wk, "Wv": wv}
        for b in range(B)
    ]
    res = bass_utils.run_bass_kernel_spmd(
        nc, in_maps, core_ids=list(range(B)), **_CACHE.get("run_kwargs", {})
    )
    _CACHE["last_results"] = res
    return np.stack([res.results[b]["out"] for b in range(B)], axis=0)


# revision 9
# speedup vs baseline: 1.3240x; 1.0255x over previous
"""Bass/Tile kernel for nn_EncoderHead: single-head encoder attention.

Per-core (data-parallel over batch B=8 across 8 NeuronCores):
  x_b [T=2048, C=768], Wq/Wk/Wv [C, H=64], mask_b [1, T] (0 = masked key)
  out_b [T, H] = softmax((x Wq)(x Wk)^T * C**-0.5, masked) @ (x Wv)

Layout strategy (all on-chip after the initial loads):
  - xT [C, T] built by PE transposes of x tiles.
  - qT, kT [H, T] = [Wq|Wk]^T @ xT in one packed matmul chain (contraction
    over c on partitions, 128 stationary columns).
  - vT [H, T] likewise, then PE-transposed to V' [T, H+1] where
    V'[s, 0:H] = v[s,:] * mask[s] and V'[s, H] = mask[s].
  - S^T[s, t] = sum_h kT[h,s] qT[h,t]   (s on partitions, t free).
  - P^T = exp(scale * S^T)  -- no max subtraction needed: logits are O(1)
    (softmax is shift-invariant; reference only shifts for stability).
  - outT'[h', t] = sum_s V'[s, h'] P^T[s, t]  accumulated over s-chunks in
    PSUM; row H is the masked softmax denominator (ones-column trick).
  - transpose outT' back, divide by denominator per-partition, DMA out.

Scheduling: emission order is software-pipelined (Tile keeps per-engine FIFO
order): x-transposes of group g+1 are emitted before projections of group g,
and S^T+exp of s-chunk js+1 before the PV matmuls of js, so the PE stream
stays dense and ACT (exp) runs back-to-back.
"""

import os
import sys

import numpy as np

_TRN_REPO = "/opt/trn_rl_repo"
if _TRN_REPO not in sys.path and os.path.isdir(_TRN_REPO):
    sys.path.insert(0, _TRN_REPO)

B, T, C, H = 8, 2048, 768, 64
P = 128  # partitions
NT = T // P      # 16 t-chunks of 128
NC = C // P      # 6 c-chunks of 128
NS = T // P      # 16 s-chunks of 128
FT = 512         # matmul moving free-dim tile
NFT = T // FT    # 4 free-dim tiles
SCALE = float(C) ** -0.5

_CACHE = {}


def _build():
    from contextlib import ExitStack

    import concourse.bass as bass  # noqa: F401
    import concourse.tile as tile
    from concourse import bacc, mybir
    from concourse.masks import make_identity

    f32 = mybir.dt.float32
    f32r = mybir.dt.float32r
    i32 = mybir.dt.int32
    EXP = mybir.ActivationFunctionType.Exp

    nc = bacc.Bacc(
        "TRN2",
        target_bir_lowering=False,
        debug=False,
        enable_asserts=False,
        num_devices=8,
    )

    x = nc.dram_tensor("x", [T, C], f32, kind="ExternalInput").ap()
    mask = nc.dram_tensor("mask", [1, T], i32, kind="ExternalInput").ap()
    wq = nc.dram_tensor("Wq", [C, H], f32, kind="ExternalInput").ap()
    wk = nc.dram_tensor("Wk", [C, H], f32, kind="ExternalInput").ap()
    wv = nc.dram_tensor("Wv", [C, H], f32, kind="ExternalInput").ap()
    out = nc.dram_tensor("out", [T, H], f32, kind="ExternalOutput").ap()

    with tile.TileContext(nc) as tc, ExitStack() as ctx:
        const = ctx.enter_context(tc.tile_pool(name="const", bufs=1))

        ident = const.tile([P, P], f32)
        make_identity(nc, ident)

        # Packed [Wq | Wk] stationary chunks: wqk_sb[:, j, 0:H] = Wq chunk j,
        # [:, j, H:2H] = Wk chunk j. One matmul chain produces qT and kT.
        wqk_st = const.tile([P, NC, 2 * H], f32)
        nc.scalar.dma_start(
            out=wqk_st[:, :, 0:H], in_=wq.rearrange("(n p) h -> p n h", p=P)
        )
        nc.scalar.dma_start(
            out=wqk_st[:, :, H : 2 * H],
            in_=wk.rearrange("(n p) h -> p n h", p=P),
        )
        wqk_sb = const.tile([P, NC, 2 * H], f32r)
        nc.gpsimd.tensor_copy(wqk_sb, wqk_st)

        wv_st = const.tile([P, NC, H], f32)
        nc.scalar.dma_start(
            out=wv_st, in_=wv.rearrange("(n p) h -> p n h", p=P)
        )
        wv_sb = const.tile([P, NC, H], f32r)
        nc.gpsimd.tensor_copy(wv_sb, wv_st)

        # mask as per-partition column per s-chunk: msk_f[p, n] = mask[n*P+p].
        # Load [16,128] natural (contiguous rows), cast, then one PE transpose.
        msk_i = const.tile([NS, P], i32)
        nc.scalar.dma_start(
            out=msk_i, in_=mask.rearrange("a (n p) -> (a n) p", p=P)
        )
        msk_n = const.tile([NS, P], f32)
        nc.vector.tensor_copy(msk_n, msk_i)
        msk_f = const.tile([P, NS], f32)

        xT_sb = const.tile([P, NC, T], f32r)      # 48KB/partition
        qT_sb = const.tile([H, T], f32r)
        kT_sb = const.tile([H, T], f32r)
        vp_sb = const.tile([P, NS, H + 1], f32r)  # V' chunks

        # ---- Phase 1+2: load x, transpose, project, build V' ----
        with ExitStack() as p12:
            xin = p12.enter_context(tc.tile_pool(name="xin", bufs=4))
            pst = p12.enter_context(
                tc.tile_pool(name="pst", bufs=4, space="PSUM")
            )
            psp = p12.enter_context(
                tc.tile_pool(name="psp", bufs=2, space="PSUM")
            )
            psv = p12.enter_context(
                tc.tile_pool(name="psv", bufs=2, space="PSUM")
            )
            vT_sb = p12.enter_context(tc.tile_pool(name="vts", bufs=1)).tile(
                [H, T], f32
            )

            pm = pst.tile([P, 512], f32, tag="pt", name="pm")
            nc.tensor.transpose(pm[:, 0:NS], msk_n, ident[0:NS, 0:NS])
            nc.vector.tensor_copy(msk_f, pm[:, 0:NS])

            def emit_chunks(g):
                for k in range(4):
                    it = 4 * g + k
                    x_tile = xin.tile([P, C], f32, name="x_tile")
                    nc.sync.dma_start(
                        out=x_tile, in_=x[it * P : (it + 1) * P, :]
                    )
                    # 6 transposed [128,128] blocks packed into 2 PSUM banks,
                    # then 2 strided copies into the xT c-planes.
                    for gg, nblk in ((0, 4), (4, 2)):
                        pt = pst.tile([P, 512], f32, tag="pt", name="pt")
                        for b in range(nblk):
                            jc = gg + b
                            nc.tensor.transpose(
                                pt[:, b * P : (b + 1) * P],
                                x_tile[:, jc * P : (jc + 1) * P],
                                ident,
                            )
                        nc.any.tensor_copy(
                            xT_sb[:, gg : gg + nblk, it * P : (it + 1) * P],
                            pt[:, 0 : nblk * P].rearrange(
                                "p (c t) -> p c t", c=nblk
                            ),
                        )

            def emit_proj(tt):
                # q,k packed: psum rows 0:H = qT tile, H:2H = kT tile
                pp = psp.tile([P, FT], f32, tag="pp", name="pp")
                for jc in range(NC):
                    nc.tensor.matmul(
                        pp,
                        lhsT=wqk_sb[:, jc, :],
                        rhs=xT_sb[:, jc, tt * FT : (tt + 1) * FT],
                        start=(jc == 0),
                        stop=(jc == NC - 1),
                    )
                nc.any.tensor_copy(
                    qT_sb[:, tt * FT : (tt + 1) * FT], pp[0:H, :]
                )
                nc.any.tensor_copy(
                    kT_sb[:, tt * FT : (tt + 1) * FT], pp[H : 2 * H, :]
                )
                pv_ps = psp.tile([H, FT], f32, tag="pp", name="pv_ps")
                for jc in range(NC):
                    nc.tensor.matmul(
                        pv_ps,
                        lhsT=wv_sb[:, jc, :],
                        rhs=xT_sb[:, jc, tt * FT : (tt + 1) * FT],
                        start=(jc == 0),
                        stop=(jc == NC - 1),
                    )
                nc.any.tensor_copy(
                    vT_sb[:, tt * FT : (tt + 1) * FT], pv_ps
                )
                # V' chunks for this t-window: v rows * mask, plus the
                # mask ones-column (masked softmax denominator trick).
                for k in range(4):
                    js = 4 * tt + k
                    pv = psv.tile([P, H], f32, tag="pv", name="pv")
                    nc.tensor.transpose(
                        pv, vT_sb[:, js * P : (js + 1) * P], ident[0:H, 0:H]
                    )
                    nc.vector.tensor_scalar_mul(
                        vp_sb[:, js, 0:H], pv, msk_f[:, js : js + 1]
                    )
                    nc.vector.tensor_copy(
                        vp_sb[:, js, H : H + 1], msk_f[:, js : js + 1]
                    )

            # software pipeline: transposes one group ahead of projections
            for g in range(NFT + 1):
                if g < NFT:
                    emit_chunks(g)
                if g >= 1:
                    emit_proj(g - 1)

        # ---- Phase 3: S^T -> exp -> accumulate outT' ----
        with ExitStack() as p3:
            pso = p3.enter_context(
                tc.tile_pool(name="pso", bufs=1, space="PSUM")
            )
            out_ps = pso.tile([H + 1, T], f32)  # 4 banks
            with ExitStack() as p3i:
                pss = p3i.enter_context(
                    tc.tile_pool(name="pss", bufs=2, space="PSUM")
                )
                pexp = p3i.enter_context(tc.tile_pool(name="pexp", bufs=3))
                from concourse.tile import add_dep_helper

                pe_tiles = {}
                st_last = {}

                def st_unit(js):
                    pe = pexp.tile([P, T], f32r, tag="pe", name="pe")
                    pe_tiles[js] = pe
                    for h in range(2):  # two [128,1024] halves
                        ps = pss.tile([P, 2 * FT], f32, tag="ps", name="ps")
                        for u in range(2):
                            tt = 2 * h + u
                            mm = nc.tensor.matmul(
                                ps[:, u * FT : (u + 1) * FT],
                                lhsT=kT_sb[:, js * P : (js + 1) * P],
                                rhs=qT_sb[:, tt * FT : (tt + 1) * FT],
                                start=True,
                                stop=True,
                            )
                        nc.scalar.activation(
                            pe[:, h * 2 * FT : (h + 1) * 2 * FT], ps, EXP,
                            scale=SCALE,
                        )
                    st_last[js] = mm

                def pv_unit(js):
                    pe = pe_tiles.pop(js)
                    gate = st_last.pop(js + 1, None)
                    for tt in range(NFT):
                        mm = nc.tensor.matmul(
                            out_ps[:, tt * FT : (tt + 1) * FT],
                            lhsT=vp_sb[:, js, :],
                            rhs=pe[:, tt * FT : (tt + 1) * FT],
                            start=(js == 0),
                            stop=(js == NS - 1),
                        )
                        if gate is not None:
                            # ordering-only edge: keep the PE queue doing
                            # S^T(js+1) before PV(js) so exp never starves
                            add_dep_helper(
                                mm.ins, gate.ins, sync=False,
                                reason="phase3 sw-pipeline order",
                            )

                st_unit(0)
                for js in range(NS):
                    if js + 1 < NS:
                        st_unit(js + 1)
                    pv_unit(js)

            # ---- Phase 4: normalize + transpose back + store ----
            with ExitStack() as p4:
                outT_sb = p4.enter_context(
                    tc.tile_pool(name="outts", bufs=1)
                ).tile([H + 1, T], f32)
                for tt in range(NFT):
                    nc.any.tensor_copy(
                        outT_sb[:, tt * FT : (tt + 1) * FT],
                        out_ps[:, tt * FT : (tt + 1) * FT],
                    )
                psf = p4.enter_context(
                    tc.tile_pool(name="psf", bufs=4, space="PSUM")
                )
                fin = p4.enter_context(tc.tile_pool(name="fin", bufs=4))
                oall = p4.enter_context(tc.tile_pool(name="oall", bufs=1)).tile(
                    [P, NT, H], f32
                )
                for it in range(NT):
                    pf = psf.tile([P, H + 1], f32, tag="pf", name="pf")
                    nc.tensor.transpose(
                        pf,
                        outT_sb[:, it * P : (it + 1) * P],
                        ident[0 : H + 1, 0 : H + 1],
                    )
                    rec = fin.tile([P, 1], f32, tag="rec", name="rec")
                    nc.vector.reciprocal(rec, pf[:, H : H + 1])
                    nc.vector.tensor_scalar_mul(
                        oall[:, it, :], pf[:, 0:H], rec
                    )
                nc.sync.dma_start(
                    out=out.rearrange("(n p) h -> p n h", p=P), in_=oall
                )

    nc.compile()
    return nc


def _get_nc():
    if "nc" not in _CACHE:
        _CACHE["nc"] = _build()
    return _CACHE["nc"]


def kernel(**inputs) -> np.ndarray:
    from concourse import bass_utils

    x = np.ascontiguousarray(np.asarray(inputs["x"], dtype=np.float32))
    mask = np.ascontiguousarray(np.asarray(inputs["mask"], dtype=np.int32))
    wq = np.ascontiguousarray(np.asarray(inputs["Wq"], dtype=np.float32))
    wk = np.ascontiguousarray(np.asarray(inputs["Wk"], dtype=np.float32))
    wv = np.ascontiguousarray(np.asarray(inputs["Wv"], dtype=np.float32))

    nc = _get_nc()
    in_maps = [
        {"x": x[b], "mask": mask[b], "Wq": wq, "Wk": wk, "Wv": wv}
        for b in range(B)
    ]
    res = bass_utils.run_bass_kernel_spmd(
        nc, in_maps, core_ids=list(range(B)), **_CACHE.get("run_kwargs", {})
    )
    _CACHE["last_results"] = res
    return np.stack([res.results[b]["out"] for b in range(B)], axis=0)
